# revision 2
# baseline (speedup 1.0000x reference)
"""Trainium2 Bass kernel for nn_ClassificationHead (MetaOptNet-Ridge head).

Per task t (256 total): K = S_t S_t^T + 50 I  (25x25);  X = 2 K^{-1} Y_t;
W = S_t^T X (640x5);  logits_t = scale * Q_t W  (300x5).

Strategy (8 NeuronCores, pure task parallelism, 32 tasks/core):
  - tasks grouped 5-at-a-time into 125x125 block-diagonal systems
  - K^{-1} via Newton-Schulz: M1 = 2aI - a^2 K closed form, 2 bf16 Newton
    iterations, then X via 2 fp32 iterative-refinement steps (validated to
    ~1e-6 X error in numpy)
  - Q streamed HBM->SBUF with fp32->bf16 cast DMA, transposed to [D, NQ]
    layout on the TensorEngine (bf16), logits^T = W^T Q^T on TensorEngine
  - device emits logits^T [5, 300] per task; host transposes on gather
"""

import numpy as np
import ml_dtypes

import concourse.bass as bass
import concourse.tile as tile
from concourse import bacc, mybir
from concourse.bass import MemorySpace, ds
from concourse.bass_utils import run_bass_kernel_spmd

F32 = mybir.dt.float32
BF16 = mybir.dt.bfloat16
NPBF16 = ml_dtypes.bfloat16

# problem shapes (hardcoded per contract)
T, NQ, NS, D, W = 256, 300, 25, 640, 5
CORES = 8
TPC = T // CORES          # 32 tasks per core
GT = 5                    # tasks per block-diag group
G = (TPC + GT - 1) // GT  # 7 groups (last group padded with 3 dummy tasks)
PT = G * GT               # 35 padded tasks per core
GP = GT * NS              # 125 partitions per group
DC = D // 128             # 5 contraction chunks
QCH = [128, 128, NQ - 256]  # query chunks per task

ALPHA = 1.4e-3            # Newton-Schulz seed: K eigs in ~[433, 1016]
LAMBDA = 50.0


def build_nc():
    nc = bacc.Bacc("TRN2", target_bir_lowering=False, debug=False,
                   num_devices=CORES)

    q = nc.dram_tensor("q", [TPC, NQ, D], F32, kind="ExternalInput")
    s = nc.dram_tensor("s", [G, GP, D], F32, kind="ExternalInput")
    y32 = nc.dram_tensor("y32", [G, GP, NS], F32, kind="ExternalInput")
    y16 = nc.dram_tensor("y16", [G, GP, NS], BF16, kind="ExternalInput")
    id16 = nc.dram_tensor("id16", [128, 128], BF16, kind="ExternalInput")
    id32 = nc.dram_tensor("id32", [GP, GP], F32, kind="ExternalInput")
    twoI = nc.dram_tensor("twoI", [GP, GP], F32, kind="ExternalInput")
    t2aI = nc.dram_tensor("t2aI", [GP, GP], F32, kind="ExternalInput")
    fifI = nc.dram_tensor("fifI", [GP, GP], F32, kind="ExternalInput")
    mask = nc.dram_tensor("mask", [GP, GP], F32, kind="ExternalInput")
    o = nc.dram_tensor("o", [TPC, W, NQ], F32, kind="ExternalOutput")

    with tile.TileContext(nc) as tc:
        with (
            tc.tile_pool(name="consts", bufs=1) as consts,
            tc.tile_pool(name="grp", bufs=2) as grp,
            tc.tile_pool(name="slv", bufs=2) as slv,
            tc.tile_pool(name="qp", bufs=3) as qp,
            tc.tile_pool(name="qtp", bufs=3) as qtp,
            tc.tile_pool(name="op", bufs=3) as op,
            tc.tile_pool(name="ps_sv", bufs=3, space=MemorySpace.PSUM) as ps_sv,
            tc.tile_pool(name="ps_qt", bufs=3, space=MemorySpace.PSUM) as ps_qt,
            tc.tile_pool(name="ps_lg", bufs=2, space=MemorySpace.PSUM) as ps_lg,
        ):
            c_id16 = consts.tile([128, 128], BF16)
            nc.sync.dma_start(out=c_id16, in_=id16[:, :])
            c_id32 = consts.tile([GP, GP], F32)
            nc.sync.dma_start(out=c_id32, in_=id32[:, :])
            c_twoI = consts.tile([GP, GP], F32)
            nc.sync.dma_start(out=c_twoI, in_=twoI[:, :])
            c_t2aI = consts.tile([GP, GP], F32)
            nc.sync.dma_start(out=c_t2aI, in_=t2aI[:, :])
            c_fifI = consts.tile([GP, GP], F32)
            nc.sync.dma_start(out=c_fifI, in_=fifI[:, :])
            c_mask = consts.tile([GP, GP], F32)
            nc.sync.dma_start(out=c_mask, in_=mask[:, :])

            for g in range(G):
                # ---- group solve: K -> M ~ K^{-1} -> X -> W ----
                s5 = grp.tile([GP, D], F32, tag="s5")
                nc.sync.dma_start(out=s5, in_=s[g])
                y32t = grp.tile([GP, NS], F32, tag="y32")
                nc.sync.dma_start(out=y32t, in_=y32[g])
                y16t = grp.tile([GP, NS], BF16, tag="y16")
                nc.sync.dma_start(out=y16t, in_=y16[g])

                # S^T chunks [128, 125] x 5 via PE transpose
                st5 = grp.tile([128, DC, GP], F32, tag="st5")
                for c in range(DC):
                    tp = ps_sv.tile([128, GP], F32, tag="sv")
                    nc.tensor.transpose(tp, s5[:, ds(128 * c, 128)], c_id32)
                    nc.scalar.copy(out=st5[:, c, :], in_=tp)

                # cross-Gram, then mask to block-diag + 50 I
                gram = ps_sv.tile([GP, GP], F32, tag="sv")
                for c in range(DC):
                    nc.tensor.matmul(gram, st5[:, c, :], st5[:, c, :],
                                     start=(c == 0), stop=(c == DC - 1))
                k32 = slv.tile([GP, GP], F32, tag="k32")
                nc.vector.tensor_mul(k32, gram, c_mask)
                nc.vector.tensor_add(k32, k32, c_fifI)
                k16 = slv.tile([GP, GP], BF16, tag="k16")
                nc.vector.tensor_copy(out=k16, in_=k32)

                # M1 = 2aI - a^2 K, then 2 bf16 Newton-Schulz iterations
                m16 = slv.tile([GP, GP], BF16, tag="m16")
                nc.scalar.mul(out=m16, in_=k32, mul=-ALPHA * ALPHA)
                nc.vector.tensor_add(m16, m16, c_t2aI)
                for _ in range(2):
                    pp = ps_sv.tile([GP, GP], F32, tag="sv")
                    nc.tensor.matmul(pp, k16, m16)
                    r16 = slv.tile([GP, GP], BF16, tag="r16")
                    nc.vector.tensor_sub(r16, c_twoI, pp)
                    mp = ps_sv.tile([GP, GP], F32, tag="sv")
                    nc.tensor.matmul(mp, m16, r16)
                    m16 = slv.tile([GP, GP], BF16, tag="m16")
                    nc.vector.tensor_copy(out=m16, in_=mp)

                # X0 = M Y, then 2 fp32 iterative-refinement steps
                xp = ps_sv.tile([GP, NS], F32, tag="sv")
                nc.tensor.matmul(xp, m16, y16t)
                xf = slv.tile([GP, NS], F32, tag="xf")
                nc.vector.tensor_copy(out=xf, in_=xp)
                for _ in range(2):
                    rp = ps_sv.tile([GP, NS], F32, tag="sv")
                    nc.tensor.matmul(rp, k32, xf)
                    r16s = slv.tile([GP, NS], BF16, tag="r16s")
                    nc.vector.tensor_sub(r16s, y32t, rp)
                    dxp = ps_sv.tile([GP, NS], F32, tag="sv")
                    nc.tensor.matmul(dxp, m16, r16s)
                    nc.vector.tensor_add(xf, xf, dxp)

                # W5[:, c, 5j:5j+5] = (S_t^T X_t) rows for d-chunk c, task j
                w5 = slv.tile([128, DC, NS], BF16, tag="w5")
                for c in range(DC):
                    wp = ps_sv.tile([128, NS], F32, tag="sv")
                    nc.tensor.matmul(wp, s5[:, ds(128 * c, 128)], xf)
                    nc.scalar.copy(out=w5[:, c, :], in_=wp)

                # ---- per-task logits ----
                for j in range(GT):
                    t = g * GT + j
                    if t >= TPC:
                        continue
                    qsb = qp.tile([128, 3, D], BF16, tag="qsb")
                    for qc in range(3):
                        qn = QCH[qc]
                        nc.gpsimd.dma_start(
                            out=qsb[:qn, qc, :],
                            in_=q[t, ds(128 * qc, qn), :])  # fp32->bf16 cast

                    qt_sb = qtp.tile([128, DC, NQ], BF16, tag="qt")
                    for c in range(DC):
                        qtps = ps_qt.tile([128, NQ], BF16, tag="qt")
                        for qc in range(3):
                            qn = QCH[qc]
                            nc.tensor.transpose(
                                qtps[:, ds(128 * qc, qn)],
                                qsb[:qn, qc, ds(128 * c, 128)],
                                c_id16[:qn, :qn])
                        if (t * DC + c) % 2 == 0:
                            nc.scalar.copy(out=qt_sb[:, c, :], in_=qtps)
                        else:
                            nc.vector.tensor_copy(out=qt_sb[:, c, :], in_=qtps)

                    lgp = ps_lg.tile([W, NQ], F32, tag="lg")
                    for c in range(DC):
                        nc.tensor.matmul(lgp, w5[:, c, ds(W * j, W)],
                                         qt_sb[:, c, :],
                                         start=(c == 0), stop=(c == DC - 1))
                    lgt = op.tile([W, NQ], F32, tag="lgt")
                    nc.scalar.copy(out=lgt, in_=lgp)
                    nc.sync.dma_start(out=o[t], in_=lgt)

    nc.compile()
    return nc


def _host_inputs(query, support, scale, support_labels):
    """Build the 8 per-core input maps (host-side shard + layout prep)."""
    scale_v = float(np.asarray(scale).reshape(-1)[0])
    labels = np.asarray(support_labels).astype(np.int64)

    ident128 = np.eye(128, dtype=np.float32)
    mask = np.zeros((GP, GP), dtype=np.float32)
    for j in range(GT):
        mask[j * NS:(j + 1) * NS, j * NS:(j + 1) * NS] = 1.0
    consts = {
        "id16": ident128.astype(NPBF16),
        "id32": ident128[:GP, :GP].copy(),
        "twoI": (2.0 * ident128[:GP, :GP]),
        "t2aI": (2.0 * ALPHA * ident128[:GP, :GP]),
        "fifI": (LAMBDA * ident128[:GP, :GP]),
        "mask": mask,
    }

    in_maps = []
    for core in range(CORES):
        t0 = core * TPC
        q_core = np.ascontiguousarray(query[t0:t0 + TPC]).astype(np.float32)
        s_core = np.zeros((G, GP, D), dtype=np.float32)
        y_core = np.zeros((G, GP, NS), dtype=np.float32)
        for tl in range(TPC):
            g, j = tl // GT, tl % GT
            s_core[g, j * NS:(j + 1) * NS, :] = support[t0 + tl]
            lab = labels[t0 + tl]
            y_core[g, j * NS + np.arange(NS), j * W + lab] = 2.0 * scale_v
        in_maps.append({
            "q": q_core,
            "s": s_core,
            "y32": y_core,
            "y16": y_core.astype(NPBF16),
            **consts,
        })
    return in_maps


_NC_CACHE = {}


def _get_nc():
    if "nc" not in _NC_CACHE:
        _NC_CACHE["nc"] = build_nc()
    return _NC_CACHE["nc"]


def kernel(query, support, scale, support_labels, n_way=5, n_shot=5, **_):
    assert int(n_way) == W and np.asarray(query).shape == (T, NQ, D)
    nc = _get_nc()
    in_maps = _host_inputs(query, support, scale, support_labels)
    res = run_bass_kernel_spmd(nc, in_maps, core_ids=list(range(CORES)))
    # gather: per-core [32, 5, 300] -> [256, 300, 5]
    full = np.concatenate([r["o"] for r in res.results], axis=0)
    return np.ascontiguousarray(full.transpose(0, 2, 1)).astype(np.float32)
